# revision 1
# baseline (speedup 1.0000x reference)
"""DN4 retrieval-kNN kernel for Trainium2 (8 NeuronCores, SPMD, no collectives).

Sharding: data-parallel over the 13230 flattened query-descriptor rows
(1654 rows -> 13 partition-tiles per core); the 5x2205-descriptor support
bank is replicated. Host adds the per-core partial (query, way) sums.

Per core:
  - support descriptors L2-normalized via diag-scaled PE transposes
    (rhs = diag(1/|y|) built by gpsimd.affine_select from a broadcast AP;
    norms: DVE squares+accum -> ACT sqrt -> DVE reciprocal)
  - query descriptors transposed raw; 1/|x| folded in AFTER top-k
    (a positive per-row scale preserves top-k selection)
  - sim = zqT.T @ desc on the PE in float32r (full-rate fp32, ~1e-4 rounding;
    produced by ACT copies which round PSUM fp32 -> SBUF float32r)
  - per-row top-8 via DVE max8 -- the pacing engine: 65 x (2205+58) cycles
    @0.96GHz ~= 153us; ACT PSUM->SBUF sim copies run just under it
  - main loop is way-outer / tile-inner so each way's descriptor bank is
    needed ~31us after the previous one; way-0's norm chain runs through
    psA's idle slots in the prologue (first max8 ~20us in), the remaining
    ways' transposes + GPSIMD diag builds trickle in the background
  - (query-slot, way) means via 13 tiny PE matmuls with a host-built
    row->slot mask (1/1323 entries) after the main loop
"""
import os
import sys

import numpy as np

for _p in ('/opt/trn_rl_repo', '/root/.axon_site/_ro/trn_rl_repo'):
    if os.path.isdir(_p) and _p not in sys.path:
        sys.path.insert(0, _p)

WAYS, SHOTS, Q = 5, 5, 30
C, HW = 128, 441
K = 3
NWAY = SHOTS * HW            # 2205 support descriptors per way
ND = WAYS * NWAY             # 11025
DT = 87                      # support descriptor row-tiles of 128
ND_PAD = DT * 128            # 11136
NCORES = 8
TROWS = Q * HW               # 13230 query-descriptor rows in total
RPC = (TROWS + NCORES - 1) // NCORES   # 1654 rows per core
MT = (RPC + 127) // 128      # 13 m-tiles per core
M_PAD = MT * 128             # 1664
SLOTS = 8                    # local query slots a core's rows can span (<=5)

CHUNKS_A = [(0, 512), (512, 512)]
CHUNKS_B = [(1024, 512), (1536, 512), (2048, 157)]

SG = 8                       # desc tiles per norm group
TG = 4                       # desc tiles per transpose PSUM tile (1 bank)
NG = (DT + SG - 1) // SG     # 11 norm groups

# desc norm groups needed before way w of the main loop can run:
# way w covers tiles floor(2205w/128) .. ceil(2205(w+1)/128)-1
def _groups_for_way(w):
    lo = (NWAY * w) // 128
    hi = (NWAY * (w + 1) + 127) // 128 - 1
    return lo // SG, hi // SG

_CACHE = {}


def _build_program():
    import concourse.bacc as bacc
    import concourse.mybir as mybir
    from concourse import tile

    dt = mybir.dt
    AF = mybir.ActivationFunctionType
    ALU = mybir.AluOpType
    AX = mybir.AxisListType

    nc = bacc.Bacc('TRN2', target_bir_lowering=False, debug=False)

    d_desc = nc.dram_tensor('desc', [128, DT * C], dt.float32, kind='ExternalInput')
    d_zq = nc.dram_tensor('zq', [128, MT * C], dt.float32, kind='ExternalInput')
    d_amask = nc.dram_tensor('amask', [128, MT * SLOTS], dt.float32, kind='ExternalInput')
    d_ident = nc.dram_tensor('ident', [128, 128], dt.float32, kind='ExternalInput')
    d_out = nc.dram_tensor('scores', [SLOTS, WAYS], dt.float32, kind='ExternalOutput')

    with tile.TileContext(nc) as tc:
        with tc.tile_pool(name='persist', bufs=1) as pp, \
             tc.tile_pool(name='work', bufs=3) as wp, \
             tc.tile_pool(name='sim', bufs=3) as simp:

            desc3 = pp.tile([128, DT, C], dt.float32)
            ddiag = pp.tile([128, DT, C], dt.float32)
            D_r = pp.tile([128, DT, C], dt.float32r)
            zq3 = pp.tile([128, MT, C], dt.float32)
            ZQ_r = pp.tile([128, MT, C], dt.float32r)
            amask3 = pp.tile([128, MT, SLOTS], dt.float32)
            ident = pp.tile([128, 128], dt.float32)
            ssd = pp.tile([128, DT], dt.float32)
            rnd = pp.tile([128, DT], dt.float32)
            ssq = pp.tile([128, MT], dt.float32)
            rnq = pp.tile([128, MT], dt.float32)
            junk_gp = pp.tile([128, C], dt.float32)
            junk_act = pp.tile([128, C], dt.float32)
            junk_dv = pp.tile([128, C], dt.float32)
            junk_dve = pp.tile([128, K], dt.float32)
            sqd = pp.tile([128, DT], dt.float32)
            sqq = pp.tile([128, MT], dt.float32)
            tiny = pp.tile([128, 1], dt.float32)
            stvals = pp.tile([128, WAYS, MT], dt.float32)
            scsb = pp.tile([SLOTS, MT, WAYS], dt.float32)
            scout = pp.tile([SLOTS, WAYS], dt.float32)

            # ---- input DMAs (critical-path slices first) ----
            half = (MT // 2) * C
            nc.sync.dma_start(desc3[:, 0:SG, :], d_desc[:, 0:SG * C])
            nc.sync.dma_start(zq3[:, 0:MT // 2, :], d_zq[:, 0:half])
            nc.sync.dma_start(ident[:], d_ident[:])
            for g in range(SG, 3 * SG, SG):
                nc.sync.dma_start(desc3[:, g:g + SG, :],
                                  d_desc[:, g * C:(g + SG) * C])
            nc.sync.dma_start(zq3[:, MT // 2:MT, :], d_zq[:, half:MT * C])
            for g in range(3 * SG, DT, SG):
                ge = min(g + SG, DT)
                nc.sync.dma_start(desc3[:, g:ge, :], d_desc[:, g * C:ge * C])
            nc.sync.dma_start(amask3[:], d_amask[:])

            nc.gpsimd.memset(tiny[:], 1e-24)

            with tc.tile_pool(name='pst', bufs=1, space='PSUM') as pst, \
                 tc.tile_pool(name='psA', bufs=2, space='PSUM') as psA, \
                 tc.tile_pool(name='psB', bufs=1, space='PSUM') as psB:

                def norm_group(gi, with_affine=True):
                    """Norms for desc tiles [gi*SG, gi*SG+SG): squares (DVE) ->
                    sqrt (ACT) -> recip (DVE) -> diag tiles (GPSIMD)."""
                    g = gi * SG
                    ge = min(g + SG, DT)
                    for t in range(g, ge):
                        # GPSIMD can't run ALU ops on HW; squares live on DVE
                        # in the DMA-bound prologue
                        nc.vector.scalar_tensor_tensor(
                            junk_dv[:], desc3[:, t, :], 1.0, desc3[:, t, :],
                            op0=ALU.mult, op1=ALU.mult,
                            accum_out=ssd[:, t:t + 1])
                    nc.scalar.activation(sqd[:, g:ge], ssd[:, g:ge], AF.Sqrt,
                                         bias=tiny[:])
                    nc.vector.reciprocal(rnd[:, g:ge], sqd[:, g:ge])
                    if with_affine:
                        affine(g, ge)

                def affine(g, ge):
                    # ddiag[p, t, f] = rnd[p, t] if p == f else 0 (gpsimd)
                    rn_b = rnd[:, g:ge].unsqueeze(2).broadcast_to([128, ge - g, C])
                    nc.gpsimd.affine_select(
                        ddiag[:, g:ge, :], rn_b, pattern=[[0, ge - g], [-1, C]],
                        compare_op=ALU.is_equal, fill=0.0,
                        base=0, channel_multiplier=1)

                def build_group(gi, early=False):
                    """Diag-scaled transposes + PSUM->SBUF copies for a group.
                    Early groups borrow psA's idle 2-bank slots (bufs=2) so the
                    way-0 prologue pipelines; background groups trickle through
                    the single-bank pst pool."""
                    g = gi * SG
                    ge = min(g + SG, DT)
                    for u in range(g, ge, TG):
                        ue = min(u + TG, ge)
                        if early:
                            pt = psA.tile([128, TG, C], dt.float32, tag='pa')
                        else:
                            pt = pst.tile([128, TG, C], dt.float32, tag='pt')
                        for j in range(ue - u):
                            t = u + j
                            nc.tensor.matmul(pt[:, j, :], desc3[:, t, :],
                                             ddiag[:, t, :], start=True, stop=True)
                        nc.scalar.activation(D_r[:, u:ue, :], pt[:, 0:ue - u, :],
                                             AF.Copy)

                def zq_unit(g):
                    ge = min(g + TG, MT)
                    pt = pst.tile([128, TG, C], dt.float32, tag='pt')
                    for j in range(ge - g):
                        nc.tensor.matmul(pt[:, j, :], zq3[:, g + j, :], ident[:],
                                         start=True, stop=True)
                    nc.scalar.activation(ZQ_r[:, g:ge, :], pt[:, 0:ge - g, :], AF.Copy)

                zq_unit(0)  # t=0..3 only; later units interleave into way 0

                # ---- way-0 desc chain first (the latency-critical path) ----
                glo0, ghi0 = _groups_for_way(0)
                done = set()
                for gi in range(glo0, ghi0 + 1):
                    norm_group(gi, with_affine=False)
                affine(glo0 * SG, min((ghi0 + 1) * SG, DT))  # one batched op
                for g in range(TG, MT, TG):
                    zq_unit(g)  # remaining query transposes, off the hot loop
                for gi in range(glo0, ghi0 + 1):
                    build_group(gi, early=True)
                    done.add(gi)
                # query norms (only needed by the epilogue rnq fold)
                for t in range(MT):
                    nc.vector.scalar_tensor_tensor(
                        junk_dv[:], zq3[:, t, :], 1.0, zq3[:, t, :],
                        op0=ALU.mult, op1=ALU.mult, accum_out=ssq[:, t:t + 1])
                nc.scalar.activation(sqq[:], ssq[:], AF.Sqrt, bias=tiny[:])
                nc.vector.reciprocal(rnq[:], sqq[:])
                # remaining descriptor norms (DMA-paced background)
                for gi in range(NG):
                    if gi not in (0, 1, 2):
                        norm_group(gi)

                # ---- main loop: way-outer / tile-inner ----
                Dflat = D_r[:].rearrange("p t c -> p (t c)")
                for w in range(WAYS):
                    base = w * NWAY
                    m8big = wp.tile([128, MT, 8], dt.float32, tag='m8')
                    for t in range(MT):
                        lhsT = ZQ_r[:, t, :]
                        pa = psA.tile([128, 1024], dt.float32, tag='pa')
                        pb = psB.tile([128, 1181], dt.float32, tag='pb')
                        for off, sz in CHUNKS_A:
                            nc.tensor.matmul(pa[:, off:off + sz], lhsT,
                                             Dflat[:, base + off:base + off + sz],
                                             start=True, stop=True)
                        for off, sz in CHUNKS_B:
                            if sz % 2:  # ragged tail: odd N fails fp32r ISA check
                                nc.tensor.matmul(
                                    pb[:, off - 1024:off - 1024 + sz],
                                    lhsT.bitcast(dt.float32),
                                    Dflat[:, base + off:base + off + sz].bitcast(dt.float32),
                                    start=True, stop=True)
                            else:
                                nc.tensor.matmul(pb[:, off - 1024:off - 1024 + sz],
                                                 lhsT,
                                                 Dflat[:, base + off:base + off + sz],
                                                 start=True, stop=True)
                        sim = simp.tile([128, NWAY], dt.float32, tag='sim')
                        nc.scalar.activation(sim[:, 0:1024], pa[:], AF.Copy)
                        nc.scalar.activation(sim[:, 1024:NWAY], pb[:], AF.Copy)
                        nc.vector.max(m8big[:, t, :], sim[:])
                    # per-way top-3 sums for all tiles in one reduce
                    nc.vector.reduce_sum(stvals[:, w, :], m8big[:, :, 0:K],
                                         axis=AX.X)
                    # emit the NEXT way's transposes right after this way's
                    # matmuls -- they execute during way w+1's 33us window
                    if w + 1 < WAYS:
                        glo, ghi = _groups_for_way(w + 1)
                        for gi in range(glo, ghi + 1):
                            if gi not in done:
                                build_group(gi)
                                done.add(gi)

            # ---- fold m-rows into (query, way) scores ----
            rq_b = rnq[:].unsqueeze(1).broadcast_to([128, WAYS, MT])
            nc.vector.tensor_tensor(stvals[:], stvals[:], rq_b, op=ALU.mult)
            with tc.tile_pool(name='psS', bufs=1, space='PSUM') as psS:
                scps = psS.tile([SLOTS, MT, WAYS], dt.float32)
                for t in range(MT):
                    nc.tensor.matmul(scps[0:SLOTS, t, :], amask3[:, t, :],
                                     stvals[:, :, t], start=True, stop=True)
                nc.scalar.activation(scsb[:], scps[:], AF.Copy)
            nc.vector.reduce_sum(scout[:], scsb[:].transpose([0, 2, 1]), axis=AX.X)
            nc.sync.dma_start(d_out[:], scout[:])

    nc.finalize()
    return nc


def _host_prep(support_images, support_labels, query_images):
    support_images = np.ascontiguousarray(np.asarray(support_images, np.float32))
    support_labels = np.asarray(support_labels, np.float32)
    query_images = np.ascontiguousarray(np.asarray(query_images, np.float32))

    labels = np.argmax(support_labels, axis=1)
    order = np.argsort(labels, kind='stable')
    sup = support_images[order].reshape(WAYS * SHOTS, C, HW)

    desc_byrow = sup.transpose(0, 2, 1).reshape(ND, C)
    desc_byrow = np.concatenate(
        [desc_byrow, np.zeros((ND_PAD - ND, C), np.float32)], 0)
    desc_dev = desc_byrow.reshape(DT, 128, C).transpose(1, 0, 2).reshape(128, DT * C)
    desc_dev = np.ascontiguousarray(desc_dev)

    # flat query-descriptor rows [13230, C], row r = (q = r//441, hw = r%441)
    zq_flat = query_images.reshape(Q, C, HW).transpose(0, 2, 1).reshape(TROWS, C)
    zq_devs, amask_devs = [], []
    for core in range(NCORES):
        r0 = core * RPC
        zb = zq_flat[r0:r0 + RPC]
        zb = np.concatenate(
            [zb, np.zeros((M_PAD - zb.shape[0], C), np.float32)], 0)
        zq_devs.append(np.ascontiguousarray(
            zb.reshape(MT, 128, C).transpose(1, 0, 2).reshape(128, MT * C)))
        q0 = r0 // HW
        amask = np.zeros((128, MT, SLOTS), np.float32)
        lr = np.arange(MT * 128)
        r = r0 + lr
        valid = (lr < RPC) & (r < TROWS)
        amask[lr[valid] % 128, lr[valid] // 128, (r[valid] // HW) - q0] = \
            1.0 / (HW * K)
        amask_devs.append(np.ascontiguousarray(amask.reshape(128, MT * SLOTS)))
    ident = np.ascontiguousarray(np.eye(128, dtype=np.float32))
    return desc_dev, zq_devs, amask_devs, ident


def kernel(support_images, support_labels, query_images):
    from concourse import bass_utils

    if 'nc' not in _CACHE:
        _CACHE['nc'] = _build_program()
    nc = _CACHE['nc']

    desc_dev, zq_devs, amask_devs, ident = _host_prep(
        support_images, support_labels, query_images)

    in_maps = [{'desc': desc_dev, 'zq': zq_devs[c],
                'amask': amask_devs[c], 'ident': ident} for c in range(NCORES)]
    try:
        res = bass_utils.run_bass_kernel_spmd(
            nc, in_maps, core_ids=list(range(NCORES)))
    except Exception:
        # transient NRT/tunnel failures happen; one retry
        import time
        time.sleep(2.0)
        res = bass_utils.run_bass_kernel_spmd(
            nc, in_maps, core_ids=list(range(NCORES)))
    scores = np.zeros((Q, WAYS), np.float32)
    for c in range(NCORES):
        q0 = (c * RPC) // HW
        part = res.results[c]['scores']
        for s in range(SLOTS):
            if q0 + s < Q:
                scores[q0 + s] += part[s]
    return scores.astype(np.float32)



# revision 3
# speedup vs baseline: 1.2511x; 1.2511x over previous
"""DN4 retrieval-kNN kernel for Trainium2 (8 NeuronCores, SPMD, no collectives).

v3: relu-fold. Host prepares the replicated class-descriptor bank (grouped,
L2-normalized, transposed to [C, n]) with each way's 2208 padded columns
stored as [delta | b]: delta_j = d_j - d_{j+1104}, b_j = d_{j+1104}. Then
max(a_j, b_j) = b_j + relu(delta-sim) lets the ACT engine do the first fold
level of the top-k with a Relu activation instead of the DVE.

Per core (1654 of 13230 query rows; 13 m-tiles), per (way, m-tile) unit:
  - 2 halves x 3 fp16 matmuls -> psum fp32 [128,1104] (delta-sims, b-sims)
  - ~90% of units (ACT-path): ACT relu(psumD)->r, ACT copy(psumB)->sb (fp16),
    DVE f1 = r + sb (2x fp16 TT-add == first max-fold), DVE fold 1104->552,
    DVE max8 over the 552 4-column group-maxes
  - rest (DVE-path, balances engines): DVE STT (psumD max 0) add psumB -> f1,
    then fold + max8
  - top-3 of group-maxes == top-3 exact unless >=2 of the top-3 share a
    group (P ~ 3/552 per row; measured 2.9e-4 rel err vs 2e-2 tolerance)
  - queries host-pre-transposed; 1/|q| computed on device and folded in
    after top-k (positive row scale preserves selection)
"""
import os
import sys

import numpy as np

for _p in ('/opt/trn_rl_repo', '/root/.axon_site/_ro/trn_rl_repo'):
    if os.path.isdir(_p) and _p not in sys.path:
        sys.path.insert(0, _p)

WAYS, SHOTS, Q = 5, 5, 30
C, HW = 128, 441
K = 3
NWAY = SHOTS * HW            # 2205 support descriptors per way
WPAD = 2208                  # per-way padded width (3 zero descriptors)
HALF = WPAD // 2             # 1104
ND = WAYS * WPAD             # 11040
DT = 87                      # bank column-tiles of 128
ND_PAD = DT * 128            # 11136
NCORES = 8
TROWS = Q * HW               # 13230 query-descriptor rows in total
RPC = (TROWS + NCORES - 1) // NCORES   # 1654 rows per core
MT = (RPC + 127) // 128      # 13 m-tiles per core
M_PAD = MT * 128             # 1664
SLOTS = 8                    # local query slots a core's rows can span (<=5)

CHUNKS = [(0, 512), (512, 512), (1024, 80)]   # matmul chunks per half

# one packed input tensor, 3 staged dma_starts (each ~2.5us fixed):
# [zqt_t0 | bank_way0 | zqt_rest | bank_rest | zq3 | amask16]
OFF_ZQT0 = 0
OFF_BANK0 = OFF_ZQT0 + 128
OFF_ZQTR = OFF_BANK0 + WPAD
OFF_BANKR = OFF_ZQTR + (MT - 1) * 128
OFF_AM = OFF_BANKR + (ND_PAD - WPAD)
BLOB = OFF_AM + 2 * MT * SLOTS

_CACHE = {}


def _build_program():
    import concourse.bacc as bacc
    import concourse.mybir as mybir
    from concourse import tile

    dt = mybir.dt
    AF = mybir.ActivationFunctionType
    ALU = mybir.AluOpType
    AX = mybir.AxisListType

    nc = bacc.Bacc('TRN2', target_bir_lowering=False, debug=False)

    d_blob = nc.dram_tensor('blob', [128, BLOB], dt.float16, kind='ExternalInput')
    d_out = nc.dram_tensor('scores', [SLOTS, WAYS], dt.float32, kind='ExternalOutput')

    with tile.TileContext(nc) as tc:
        with tc.tile_pool(name='persist', bufs=1) as pp, \
             tc.tile_pool(name='work', bufs=3) as wp:

            blob = pp.tile([128, BLOB], dt.float16)

            def zqt(t):
                if t == 0:
                    return blob[:, OFF_ZQT0:OFF_ZQT0 + 128]
                o = OFF_ZQTR + (t - 1) * 128
                return blob[:, o:o + 128]

            def bankw(w, lo, hi):
                if w == 0:
                    return blob[:, OFF_BANK0 + lo:OFF_BANK0 + hi]
                o = OFF_BANKR + (w - 1) * WPAD
                return blob[:, o + lo:o + hi]

            amask3 = blob[:, OFF_AM:OFF_AM + 2 * MT * SLOTS].bitcast(
                dt.float32).rearrange('p (t s) -> p t s', t=MT)
            junk16 = pp.tile([128, C], dt.float16)
            scsb = pp.tile([SLOTS, MT, WAYS], dt.float32)
            scout = pp.tile([SLOTS, WAYS], dt.float32)

            # ---- input DMAs, staged so unit (0,0) starts asap ----
            nc.sync.dma_start(blob[:, 0:OFF_ZQTR], d_blob[:, 0:OFF_ZQTR])
            nc.sync.dma_start(blob[:, OFF_ZQTR:OFF_BANKR],
                              d_blob[:, OFF_ZQTR:OFF_BANKR])
            nc.sync.dma_start(blob[:, OFF_BANKR:BLOB],
                              d_blob[:, OFF_BANKR:BLOB])

            nc.gpsimd.memset(junk16[:], 0.0)

            with tc.tile_pool(name='ps', bufs=1, space='PSUM') as ps, \
                 tc.tile_pool(name='psS', bufs=1, space='PSUM') as psS:

                scps = psS.tile([SLOTS, MT, WAYS], dt.float32)

                # warm the PE through its p-state ramp during the input DMA
                # (full clock needs ~3us of continuous execution)
                warm = ps.tile([128, C], dt.float32, tag='pD')
                for _ in range(28):
                    nc.tensor.matmul(warm[:], junk16[:], junk16[:],
                                     start=True, stop=True)

                # ---- main loop: way-outer / tile-inner ----
                for w in range(WAYS):
                    m8big = wp.tile([128, MT, 8], dt.float16, tag='m8')
                    for t in range(MT):
                        f1 = wp.tile([128, HALF], dt.float16, tag='f1')
                        f2 = wp.tile([128, HALF // 2], dt.float16, tag='f2')
                        pD = ps.tile([128, HALF], dt.float32, tag='pD')
                        pB = ps.tile([128, HALF], dt.float32, tag='pB')
                        for off, sz in CHUNKS:
                            nc.tensor.matmul(
                                pD[:, off:off + sz], zqt(t),
                                bankw(w, off, off + sz),
                                start=True, stop=True)
                        for off, sz in CHUNKS:
                            nc.tensor.matmul(
                                pB[:, off:off + sz], zqt(t),
                                bankw(w, HALF + off, HALF + off + sz),
                                start=True, stop=True)
                        # f1 = b + relu(a-b) == max(a, b); the TT-add runs
                        # in the 2x fp16 DVE mode (dual-PSUM reads are not
                        # legal on HW, so ACT egresses everything)
                        r = wp.tile([128, HALF], dt.float16, tag='r')
                        sb = wp.tile([128, HALF], dt.float16, tag='sb')
                        nc.scalar.activation(r[:], pD[:], AF.Relu)
                        nc.scalar.activation(sb[:], pB[:], AF.Copy)
                        nc.vector.tensor_tensor(f1[:], r[:], sb[:],
                                                op=ALU.add)
                        nc.vector.tensor_tensor(f2[:], f1[:, 0:552],
                                                f1[:, 552:HALF], op=ALU.max)
                        nc.vector.max(m8big[:, t, :], f2[:])
                    # 1/|q| and 1/(441*3) live in the host-built amask, so
                    # each way's score column accumulates as soon as its
                    # top-3 sums exist; way 4 goes per-tile (shorter tail)
                    stv = wp.tile([128, MT], dt.float32, tag='stv')
                    nc.vector.reduce_sum(stv[:], m8big[:, :, 0:K], axis=AX.X)
                    for t in range(MT):
                        nc.tensor.matmul(scps[0:SLOTS, t, w:w + 1],
                                         amask3[:, t, :], stv[:, t:t + 1],
                                         start=True, stop=True)
                nc.scalar.activation(scsb[:], scps[:], AF.Copy)
            nc.vector.reduce_sum(scout[:], scsb[:].transpose([0, 2, 1]), axis=AX.X)
            nc.sync.dma_start(d_out[:], scout[:])

    nc.finalize()
    return nc


def _host_prep(support_images, support_labels, query_images):
    support_images = np.ascontiguousarray(np.asarray(support_images, np.float32))
    support_labels = np.asarray(support_labels, np.float32)
    query_images = np.ascontiguousarray(np.asarray(query_images, np.float32))

    labels = np.argmax(support_labels, axis=1)
    order = np.argsort(labels, kind='stable')
    sup = support_images[order].reshape(WAYS * SHOTS, C, HW)

    # replicated class-descriptor bank: grouped, fp16, L2-normalized over C
    # (norms from the fp16-rounded values the matmuls see), padded per way
    desc = sup.transpose(0, 2, 1).reshape(WAYS, NWAY, C).astype(np.float16)
    dn = np.sqrt((desc.astype(np.float32) ** 2).sum(-1, keepdims=True) + 1e-4)
    dhat = (desc.astype(np.float32) / dn)
    dpad = np.zeros((WAYS, WPAD, C), np.float32)
    dpad[:, :NWAY] = dhat
    # [delta | b] halves per way
    bankw = np.empty_like(dpad)
    bankw[:, :HALF] = dpad[:, :HALF] - dpad[:, HALF:]
    bankw[:, HALF:] = dpad[:, HALF:]
    flat = bankw.reshape(ND, C)
    flat = np.concatenate([flat, np.zeros((ND_PAD - ND, C), np.float32)], 0)
    bank_dev = flat.T.astype(np.float16)                         # [C, ND_PAD]

    # flat query-descriptor rows [13230, C], row r = (q = r//441, hw = r%441)
    zq_flat = query_images.reshape(Q, C, HW).transpose(0, 2, 1).reshape(TROWS, C)
    blob_devs = []
    for core in range(NCORES):
        r0 = core * RPC
        zb = zq_flat[r0:r0 + RPC]
        zb = np.concatenate(
            [zb, np.zeros((M_PAD - zb.shape[0], C), np.float32)], 0)
        zqt_dev = zb.T.reshape(C, MT * 128).astype(np.float16)
        # 1/|q| per padded row (from the fp16 values the matmuls see),
        # folded into the amask weights
        q16 = zb.astype(np.float16).astype(np.float32)
        qn = np.sqrt((q16 ** 2).sum(1) + 1e-4)
        q0 = r0 // HW
        amask = np.zeros((128, MT, SLOTS), np.float32)
        lr = np.arange(MT * 128)
        r = r0 + lr
        valid = (lr < RPC) & (r < TROWS)
        amask[lr[valid] % 128, lr[valid] // 128, (r[valid] // HW) - q0] = \
            1.0 / (HW * K * qn[lr[valid]])
        am16 = amask.reshape(128, MT * SLOTS).view(np.float16)
        blob = np.concatenate(
            [zqt_dev[:, 0:128], bank_dev[:, 0:WPAD], zqt_dev[:, 128:],
             bank_dev[:, WPAD:], am16], axis=1)
        blob_devs.append(np.ascontiguousarray(blob))
    return blob_devs


def kernel(support_images, support_labels, query_images):
    from concourse import bass_utils

    if 'nc' not in _CACHE:
        _CACHE['nc'] = _build_program()
    nc = _CACHE['nc']

    blob_devs = _host_prep(support_images, support_labels, query_images)

    in_maps = [{'blob': blob_devs[c]} for c in range(NCORES)]
    try:
        res = bass_utils.run_bass_kernel_spmd(
            nc, in_maps, core_ids=list(range(NCORES)))
    except Exception:
        # transient NRT/tunnel failures happen; one retry
        import time
        time.sleep(2.0)
        res = bass_utils.run_bass_kernel_spmd(
            nc, in_maps, core_ids=list(range(NCORES)))
    scores = np.zeros((Q, WAYS), np.float32)
    for c in range(NCORES):
        q0 = (c * RPC) // HW
        part = res.results[c]['scores']
        for s in range(SLOTS):
            if q0 + s < Q:
                scores[q0 + s] += part[s]
    return scores.astype(np.float32)


# revision 4
# speedup vs baseline: 1.2592x; 1.0065x over previous
"""DN4 retrieval-kNN kernel for Trainium2 (8 NeuronCores, SPMD, no collectives).

v3: relu-fold. Host prepares the replicated class-descriptor bank (grouped,
L2-normalized, transposed to [C, n]) with each way's 2208 padded columns
stored as [delta | b]: delta_j = d_j - d_{j+1104}, b_j = d_{j+1104}. Then
max(a_j, b_j) = b_j + relu(delta-sim) lets the ACT engine do the first fold
level of the top-k with a Relu activation instead of the DVE.

Per core (1654 of 13230 query rows; 13 m-tiles), per (way, m-tile) unit:
  - 2 halves x 3 fp16 matmuls -> psum fp32 [128,1104] (delta-sims, b-sims)
  - ~90% of units (ACT-path): ACT relu(psumD)->r, ACT copy(psumB)->sb (fp16),
    DVE f1 = r + sb (2x fp16 TT-add == first max-fold), DVE fold 1104->552,
    DVE max8 over the 552 4-column group-maxes
  - rest (DVE-path, balances engines): DVE STT (psumD max 0) add psumB -> f1,
    then fold + max8
  - top-3 of group-maxes == top-3 exact unless >=2 of the top-3 share a
    group (P ~ 3/552 per row; measured 2.9e-4 rel err vs 2e-2 tolerance)
  - queries host-pre-transposed; 1/|q| computed on device and folded in
    after top-k (positive row scale preserves selection)
"""
import os
import sys

import numpy as np

for _p in ('/opt/trn_rl_repo', '/root/.axon_site/_ro/trn_rl_repo'):
    if os.path.isdir(_p) and _p not in sys.path:
        sys.path.insert(0, _p)

WAYS, SHOTS, Q = 5, 5, 30
C, HW = 128, 441
K = 3
NWAY = SHOTS * HW            # 2205 support descriptors per way
WPAD = 2208                  # per-way padded width (3 zero descriptors)
HALF = WPAD // 2             # 1104
ND = WAYS * WPAD             # 11040
DT = 87                      # bank column-tiles of 128
ND_PAD = DT * 128            # 11136
NCORES = 8
TROWS = Q * HW               # 13230 query-descriptor rows in total
RPC = (TROWS + NCORES - 1) // NCORES   # 1654 rows per core
MT = (RPC + 127) // 128      # 13 m-tiles per core
M_PAD = MT * 128             # 1664
SLOTS = 8                    # local query slots a core's rows can span (<=5)

CHUNKS = [(0, 512), (512, 512), (1024, 80)]   # matmul chunks per half

# one packed input tensor, 3 staged dma_starts (each ~2.5us fixed):
# [zqt_t0 | bank_way0 | zqt_rest | bank_rest | zq3 | amask16]
OFF_ZQT0 = 0
OFF_BANK0 = OFF_ZQT0 + 128
OFF_ZQTR = OFF_BANK0 + WPAD
OFF_BANKR = OFF_ZQTR + (MT - 1) * 128
OFF_AM = OFF_BANKR + (ND_PAD - WPAD)
BLOB = OFF_AM + 2 * MT * SLOTS

_CACHE = {}


def _build_program():
    import concourse.bacc as bacc
    import concourse.mybir as mybir
    from concourse import tile

    dt = mybir.dt
    AF = mybir.ActivationFunctionType
    ALU = mybir.AluOpType
    AX = mybir.AxisListType

    nc = bacc.Bacc('TRN2', target_bir_lowering=False, debug=False)

    d_blob = nc.dram_tensor('blob', [128, BLOB], dt.float16, kind='ExternalInput')
    d_out = nc.dram_tensor('scores', [SLOTS, MT * WAYS], dt.float32,
                           kind='ExternalOutput')

    with tile.TileContext(nc) as tc:
        with tc.tile_pool(name='persist', bufs=1) as pp, \
             tc.tile_pool(name='work', bufs=3) as wp:

            blob = pp.tile([128, BLOB], dt.float16)

            def zqt(t):
                if t == 0:
                    return blob[:, OFF_ZQT0:OFF_ZQT0 + 128]
                o = OFF_ZQTR + (t - 1) * 128
                return blob[:, o:o + 128]

            def bankw(w, lo, hi):
                if w == 0:
                    return blob[:, OFF_BANK0 + lo:OFF_BANK0 + hi]
                o = OFF_BANKR + (w - 1) * WPAD
                return blob[:, o + lo:o + hi]

            amask3 = blob[:, OFF_AM:OFF_AM + 2 * MT * SLOTS].bitcast(
                dt.float32).rearrange('p (t s) -> p t s', t=MT)
            junk16 = pp.tile([128, C], dt.float16)
            scsb = pp.tile([SLOTS, MT, WAYS], dt.float32)

            # ---- input DMAs, staged so unit (0,0) starts asap ----
            nc.sync.dma_start(blob[:, 0:OFF_BANK0 + HALF],
                              d_blob[:, 0:OFF_BANK0 + HALF])
            nc.sync.dma_start(blob[:, OFF_BANK0 + HALF:OFF_ZQTR],
                              d_blob[:, OFF_BANK0 + HALF:OFF_ZQTR])
            nc.sync.dma_start(blob[:, OFF_ZQTR:OFF_BANKR],
                              d_blob[:, OFF_ZQTR:OFF_BANKR])
            nc.sync.dma_start(blob[:, OFF_BANKR:BLOB],
                              d_blob[:, OFF_BANKR:BLOB])

            nc.gpsimd.memset(junk16[:], 0.0)

            with tc.tile_pool(name='ps', bufs=1, space='PSUM') as ps, \
                 tc.tile_pool(name='psS', bufs=1, space='PSUM') as psS:

                scps = psS.tile([SLOTS, MT, WAYS], dt.float32)

                # warm the PE through its p-state ramp during the input DMA
                # (full clock needs ~3us of continuous execution)
                warm = psS.tile([128, C], dt.float32, tag='warm')
                for _ in range(16):
                    nc.tensor.matmul(warm[:], junk16[:], junk16[:],
                                     start=True, stop=True)

                # ---- main loop: way-outer / tile-inner ----
                for w in range(WAYS):
                    m8big = wp.tile([128, MT, 8], dt.float16, tag='m8')
                    for t in range(MT):
                        f1 = wp.tile([128, HALF], dt.float16, tag='f1')
                        f2 = wp.tile([128, HALF // 2], dt.float16, tag='f2')
                        pD = ps.tile([128, HALF], dt.float32, tag='pD')
                        pB = ps.tile([128, HALF], dt.float32, tag='pB')
                        for off, sz in CHUNKS:
                            nc.tensor.matmul(
                                pD[:, off:off + sz], zqt(t),
                                bankw(w, off, off + sz),
                                start=True, stop=True)
                        for off, sz in CHUNKS:
                            nc.tensor.matmul(
                                pB[:, off:off + sz], zqt(t),
                                bankw(w, HALF + off, HALF + off + sz),
                                start=True, stop=True)
                        # f1 = b + relu(a-b) == max(a, b); the TT-add runs
                        # in the 2x fp16 DVE mode (dual-PSUM reads are not
                        # legal on HW, so ACT egresses everything)
                        r = wp.tile([128, HALF], dt.float16, tag='r')
                        sb = wp.tile([128, HALF], dt.float16, tag='sb')
                        nc.scalar.activation(r[:], pD[:], AF.Relu)
                        nc.scalar.activation(sb[:], pB[:], AF.Copy)
                        nc.vector.tensor_tensor(f1[:], r[:], sb[:],
                                                op=ALU.add)
                        nc.vector.tensor_tensor(f2[:], f1[:, 0:552],
                                                f1[:, 552:HALF], op=ALU.max)
                        nc.vector.max(m8big[:, t, :], f2[:])
                    # 1/|q| and 1/(441*3) live in the host-built amask, so
                    # each way's score column accumulates as soon as its
                    # top-3 sums exist; way 4 goes per-tile (shorter tail)
                    stv = wp.tile([128, MT], dt.float32, tag='stv')
                    nc.vector.reduce_sum(stv[:], m8big[:, :, 0:K], axis=AX.X)
                    for t in range(MT):
                        nc.tensor.matmul(scps[0:SLOTS, t, w:w + 1],
                                         amask3[:, t, :], stv[:, t:t + 1],
                                         start=True, stop=True)
                nc.scalar.activation(scsb[:], scps[:], AF.Copy)
            # host sums the MT axis (and across cores) - saves a tail reduce
            nc.sync.dma_start(d_out[:], scsb[:].rearrange('s t w -> s (t w)'))

    nc.finalize()
    return nc


def _host_prep(support_images, support_labels, query_images):
    support_images = np.ascontiguousarray(np.asarray(support_images, np.float32))
    support_labels = np.asarray(support_labels, np.float32)
    query_images = np.ascontiguousarray(np.asarray(query_images, np.float32))

    labels = np.argmax(support_labels, axis=1)
    order = np.argsort(labels, kind='stable')
    sup = support_images[order].reshape(WAYS * SHOTS, C, HW)

    # replicated class-descriptor bank: grouped, fp16, L2-normalized over C
    # (norms from the fp16-rounded values the matmuls see), padded per way
    desc = sup.transpose(0, 2, 1).reshape(WAYS, NWAY, C).astype(np.float16)
    dn = np.sqrt((desc.astype(np.float32) ** 2).sum(-1, keepdims=True) + 1e-4)
    dhat = (desc.astype(np.float32) / dn)
    dpad = np.zeros((WAYS, WPAD, C), np.float32)
    dpad[:, :NWAY] = dhat
    # [delta | b] halves per way
    bankw = np.empty_like(dpad)
    bankw[:, :HALF] = dpad[:, :HALF] - dpad[:, HALF:]
    bankw[:, HALF:] = dpad[:, HALF:]
    flat = bankw.reshape(ND, C)
    flat = np.concatenate([flat, np.zeros((ND_PAD - ND, C), np.float32)], 0)
    bank_dev = flat.T.astype(np.float16)                         # [C, ND_PAD]

    # flat query-descriptor rows [13230, C], row r = (q = r//441, hw = r%441)
    zq_flat = query_images.reshape(Q, C, HW).transpose(0, 2, 1).reshape(TROWS, C)
    blob_devs = []
    for core in range(NCORES):
        r0 = core * RPC
        zb = zq_flat[r0:r0 + RPC]
        zb = np.concatenate(
            [zb, np.zeros((M_PAD - zb.shape[0], C), np.float32)], 0)
        zqt_dev = zb.T.reshape(C, MT * 128).astype(np.float16)
        # 1/|q| per padded row (from the fp16 values the matmuls see),
        # folded into the amask weights
        q16 = zb.astype(np.float16).astype(np.float32)
        qn = np.sqrt((q16 ** 2).sum(1) + 1e-4)
        q0 = r0 // HW
        amask = np.zeros((128, MT, SLOTS), np.float32)
        lr = np.arange(MT * 128)
        r = r0 + lr
        valid = (lr < RPC) & (r < TROWS)
        amask[lr[valid] % 128, lr[valid] // 128, (r[valid] // HW) - q0] = \
            1.0 / (HW * K * qn[lr[valid]])
        am16 = amask.reshape(128, MT * SLOTS).view(np.float16)
        blob = np.concatenate(
            [zqt_dev[:, 0:128], bank_dev[:, 0:WPAD], zqt_dev[:, 128:],
             bank_dev[:, WPAD:], am16], axis=1)
        blob_devs.append(np.ascontiguousarray(blob))
    return blob_devs


def kernel(support_images, support_labels, query_images):
    from concourse import bass_utils

    if 'nc' not in _CACHE:
        _CACHE['nc'] = _build_program()
    nc = _CACHE['nc']

    blob_devs = _host_prep(support_images, support_labels, query_images)

    in_maps = [{'blob': blob_devs[c]} for c in range(NCORES)]
    try:
        res = bass_utils.run_bass_kernel_spmd(
            nc, in_maps, core_ids=list(range(NCORES)))
    except Exception:
        # transient NRT/tunnel failures happen; one retry
        import time
        time.sleep(2.0)
        res = bass_utils.run_bass_kernel_spmd(
            nc, in_maps, core_ids=list(range(NCORES)))
    scores = np.zeros((Q, WAYS), np.float32)
    for c in range(NCORES):
        q0 = (c * RPC) // HW
        part = res.results[c]['scores'].reshape(SLOTS, MT, WAYS).sum(1)
        for s in range(SLOTS):
            if q0 + s < Q:
                scores[q0 + s] += part[s]
    return scores.astype(np.float32)


# revision 5
# speedup vs baseline: 1.3455x; 1.0685x over previous
"""DN4 retrieval-kNN kernel for Trainium2 (8 NeuronCores, SPMD, no collectives).

v4: relu-fold with PSUM accumulation. Host prepares the replicated
class-descriptor bank (grouped, L2-normalized, transposed to [C, n]) with
each way's 2208 padded columns stored as [delta | b]: delta_j =
d_j - d_{j+1104}, b_j = d_{j+1104}. On device, per (way, m-tile) unit:

  PE:  delta-sims -> pR psum fp32                 (q . delta, 1104 cols)
  ACT: relu(pR) -> pR IN PLACE                    (one pass, half the old)
  PE:  b-sims ACCUMULATE onto pR (start=False)    -> pR = b + relu(a-b)
                                                   = max(a, b) exactly
  DVE: max8 over the 1104 pair-maxes in pR, top-3 of those == top-3 of
       the way's 2205 sims unless >=2 of the top-3 share a pair
       (P ~ 3/1104 per row; error way under the 2e-2 tolerance)

DVE (max8 at 1 elem/lane/cycle) is the pacing engine: ~1.37us x 65 units.
Queries are host-pre-transposed; 1/|q| and 1/(441*3) live in the host-built
amask, applied by the per-way score matmuls; host sums m-tiles and cores.
"""
import os
import sys

import numpy as np

for _p in ('/opt/trn_rl_repo', '/root/.axon_site/_ro/trn_rl_repo'):
    if os.path.isdir(_p) and _p not in sys.path:
        sys.path.insert(0, _p)

WAYS, SHOTS, Q = 5, 5, 30
C, HW = 128, 441
K = 3
NWAY = SHOTS * HW            # 2205 support descriptors per way
WPAD = 2208                  # per-way padded width (3 zero descriptors)
HALF = WPAD // 2             # 1104 pairs per way
ND = WAYS * WPAD             # 11040
DT = 87                      # bank column-tiles of 128
ND_PAD = DT * 128            # 11136
NCORES = 8
TROWS = Q * HW               # 13230 query-descriptor rows in total
RPC = (TROWS + NCORES - 1) // NCORES   # 1654 rows per core
MT = (RPC + 127) // 128      # 13 m-tiles per core
M_PAD = MT * 128             # 1664
SLOTS = 8                    # local query slots a core's rows can span (<=5)

QUART = HALF // 2            # 552

# one packed input tensor, staged dma_starts (each ~2.5us fixed):
# [zqt_t0 | bank_way0 | zqt_rest | bank_rest | amask16]
OFF_ZQT0 = 0
OFF_BANK0 = OFF_ZQT0 + 128
OFF_ZQTR = OFF_BANK0 + WPAD
OFF_BANKR = OFF_ZQTR + (MT - 1) * 128
OFF_AM = OFF_BANKR + (ND_PAD - WPAD)
BLOB = OFF_AM + 2 * MT * SLOTS

_CACHE = {}


def _build_program():
    import concourse.bacc as bacc
    import concourse.mybir as mybir
    from concourse import tile

    dt = mybir.dt
    AF = mybir.ActivationFunctionType
    ALU = mybir.AluOpType
    AX = mybir.AxisListType

    nc = bacc.Bacc('TRN2', target_bir_lowering=False, debug=False)

    d_blob = nc.dram_tensor('blob', [128, BLOB], dt.float16, kind='ExternalInput')
    d_out = nc.dram_tensor('scores', [SLOTS, MT * WAYS], dt.float32,
                           kind='ExternalOutput')

    with tile.TileContext(nc) as tc:
        with tc.tile_pool(name='persist', bufs=1) as pp, \
             tc.tile_pool(name='work', bufs=3) as wp:

            blob = pp.tile([128, BLOB], dt.float16)

            def zqt(t):
                if t == 0:
                    return blob[:, OFF_ZQT0:OFF_ZQT0 + 128]
                o = OFF_ZQTR + (t - 1) * 128
                return blob[:, o:o + 128]

            def bankw(w, lo, hi):
                if w == 0:
                    return blob[:, OFF_BANK0 + lo:OFF_BANK0 + hi]
                o = OFF_BANKR + (w - 1) * WPAD
                return blob[:, o + lo:o + hi]

            amask3 = blob[:, OFF_AM:OFF_AM + 2 * MT * SLOTS].bitcast(
                dt.float32).rearrange('p (t s) -> p t s', t=MT)
            scsb = pp.tile([SLOTS, MT, WAYS], dt.float32)

            # ---- input DMAs, staged so unit (0,0) starts asap ----
            nc.sync.dma_start(blob[:, 0:OFF_BANK0 + HALF],
                              d_blob[:, 0:OFF_BANK0 + HALF])
            nc.sync.dma_start(blob[:, OFF_BANK0 + HALF:OFF_ZQTR],
                              d_blob[:, OFF_BANK0 + HALF:OFF_ZQTR])
            nc.sync.dma_start(blob[:, OFF_ZQTR:OFF_BANKR],
                              d_blob[:, OFF_ZQTR:OFF_BANKR])
            nc.sync.dma_start(blob[:, OFF_BANKR:BLOB],
                              d_blob[:, OFF_BANKR:BLOB])

            with tc.tile_pool(name='ps', bufs=2, space='PSUM') as ps, \
                 tc.tile_pool(name='psS', bufs=1, space='PSUM') as psS:

                scps = psS.tile([SLOTS, MT, WAYS], dt.float32)

                units = [(w, t) for w in range(WAYS) for t in range(MT)]
                m8bigs, pRs = {}, {}
                pending = []

                def emit_front(i):
                    w, t = units[i]
                    pR = ps.tile([128, HALF], dt.float32, tag='pR',
                                 name=f'pR_{i}')
                    pRs[i] = pR
                    # delta-sims; the in-place relus pipeline behind the
                    # matmuls chunk by chunk
                    for off, sz in ((0, 512), (512, 512), (1024, 80)):
                        nc.tensor.matmul(pR[:, off:off + sz], zqt(t),
                                         bankw(w, off, off + sz),
                                         start=True, stop=True)
                    nc.scalar.activation(pR[:], pR[:], AF.Relu)

                def emit_back(i):
                    w, t = units[i]
                    pR = pRs.pop(i)
                    if t == 0:
                        m8bigs[w] = wp.tile([128, MT, 8], dt.float32, tag='m8',
                                            name=f'm8_{w}')
                    # b-sims accumulate onto relu(delta): pR = max(a, b)
                    for off, sz in ((0, 512), (512, 512), (1024, 80)):
                        nc.tensor.matmul(pR[:, off:off + sz], zqt(t),
                                         bankw(w, HALF + off,
                                               HALF + off + sz),
                                         start=False, stop=True)
                    nc.vector.max(m8bigs[w][:, t, :], pR[:])
                    if t == MT - 1:
                        pending.append(w)

                def emit_wayend():
                    w = pending.pop(0)
                    m8big = m8bigs.pop(w)
                    stv = wp.tile([128, MT], dt.float32, tag='stv')
                    nc.vector.reduce_sum(stv[:], m8big[:, :, 0:K], axis=AX.X)
                    for tt in range(MT):
                        nc.tensor.matmul(scps[0:SLOTS, tt, w:w + 1],
                                         amask3[:, tt, :], stv[:, tt:tt + 1],
                                         start=True, stop=True)

                # software-pipelined by one stage so the in-order PE queue
                # never waits on a relu: delta(i+1) runs while b(i) waits
                for i in range(len(units) + 1):
                    if i < len(units):
                        emit_front(i)
                    if i >= 1:
                        emit_back(i - 1)
                    if pending and (i - 1 >= len(units) - 1 or
                                    (i >= 4 and units[i - 4][1] == MT - 1)):
                        emit_wayend()
                while pending:
                    emit_wayend()
                nc.scalar.activation(scsb[:], scps[:], AF.Copy)
            # host sums the MT axis (and across cores)
            nc.sync.dma_start(d_out[:], scsb[:].rearrange('s t w -> s (t w)'))

    nc.finalize()
    return nc


def _host_prep(support_images, support_labels, query_images):
    support_images = np.ascontiguousarray(np.asarray(support_images, np.float32))
    support_labels = np.asarray(support_labels, np.float32)
    query_images = np.ascontiguousarray(np.asarray(query_images, np.float32))

    labels = np.argmax(support_labels, axis=1)
    order = np.argsort(labels, kind='stable')
    sup = support_images[order].reshape(WAYS * SHOTS, C, HW)

    # replicated class-descriptor bank: grouped, fp16, L2-normalized over C
    # (norms from the fp16-rounded values the matmuls see), padded per way
    desc = sup.transpose(0, 2, 1).reshape(WAYS, NWAY, C).astype(np.float16)
    dn = np.sqrt((desc.astype(np.float32) ** 2).sum(-1, keepdims=True) + 1e-4)
    dhat = (desc.astype(np.float32) / dn)
    dpad = np.zeros((WAYS, WPAD, C), np.float32)
    dpad[:, :NWAY] = dhat
    # [delta | b] halves per way
    bankw = np.empty_like(dpad)
    bankw[:, :HALF] = dpad[:, :HALF] - dpad[:, HALF:]
    bankw[:, HALF:] = dpad[:, HALF:]
    flat = bankw.reshape(ND, C)
    flat = np.concatenate([flat, np.zeros((ND_PAD - ND, C), np.float32)], 0)
    bank_dev = flat.T.astype(np.float16)                         # [C, ND_PAD]

    # flat query-descriptor rows [13230, C], row r = (q = r//441, hw = r%441)
    zq_flat = query_images.reshape(Q, C, HW).transpose(0, 2, 1).reshape(TROWS, C)
    blob_devs = []
    for core in range(NCORES):
        r0 = core * RPC
        zb = zq_flat[r0:r0 + RPC]
        zb = np.concatenate(
            [zb, np.zeros((M_PAD - zb.shape[0], C), np.float32)], 0)
        zqt_dev = zb.T.reshape(C, MT * 128).astype(np.float16)
        # 1/|q| per padded row (from the fp16 values the matmuls see),
        # folded into the amask weights
        q16 = zb.astype(np.float16).astype(np.float32)
        qn = np.sqrt((q16 ** 2).sum(1) + 1e-4)
        q0 = r0 // HW
        amask = np.zeros((128, MT, SLOTS), np.float32)
        lr = np.arange(MT * 128)
        r = r0 + lr
        valid = (lr < RPC) & (r < TROWS)
        amask[lr[valid] % 128, lr[valid] // 128, (r[valid] // HW) - q0] = \
            1.0 / (HW * K * qn[lr[valid]])
        am16 = amask.reshape(128, MT * SLOTS).view(np.float16)
        blob = np.concatenate(
            [zqt_dev[:, 0:128], bank_dev[:, 0:WPAD], zqt_dev[:, 128:],
             bank_dev[:, WPAD:], am16], axis=1)
        blob_devs.append(np.ascontiguousarray(blob))
    return blob_devs


def kernel(support_images, support_labels, query_images):
    from concourse import bass_utils

    if 'nc' not in _CACHE:
        _CACHE['nc'] = _build_program()
    nc = _CACHE['nc']

    blob_devs = _host_prep(support_images, support_labels, query_images)

    in_maps = [{'blob': blob_devs[c]} for c in range(NCORES)]
    try:
        res = bass_utils.run_bass_kernel_spmd(
            nc, in_maps, core_ids=list(range(NCORES)))
    except Exception:
        # transient NRT/tunnel failures happen; one retry
        import time
        time.sleep(2.0)
        res = bass_utils.run_bass_kernel_spmd(
            nc, in_maps, core_ids=list(range(NCORES)))
    scores = np.zeros((Q, WAYS), np.float32)
    for c in range(NCORES):
        q0 = (c * RPC) // HW
        part = res.results[c]['scores'].reshape(SLOTS, MT, WAYS).sum(1)
        for s in range(SLOTS):
            if q0 + s < Q:
                scores[q0 + s] += part[s]
    return scores.astype(np.float32)


# revision 6
# speedup vs baseline: 1.7053x; 1.2674x over previous
"""DN4 retrieval-kNN kernel for Trainium2 (8 NeuronCores, SPMD, no collectives).

v5: relu-fold with PSUM accumulation, half-unit pipelining. Host prepares the replicated
class-descriptor bank (grouped, L2-normalized, transposed to [C, n]) with
each way's 2208 padded columns stored as [delta | b]: delta_j =
d_j - d_{j+1104}, b_j = d_{j+1104}. On device, per (way, m-tile) unit:

  PE:  delta-sims -> pR psum fp32                 (q . delta, 1104 cols)
  ACT: relu(pR) -> pR IN PLACE                    (one pass, half the old)
  PE:  b-sims ACCUMULATE onto pR (start=False)    -> pR = b + relu(a-b)
                                                   = max(a, b) exactly
  DVE: max8 over the 1104 pair-maxes in pR, top-3 of those == top-3 of
       the way's 2205 sims unless >=2 of the top-3 share a pair
       (P ~ 3/1104 per row; error way under the 2e-2 tolerance)

DVE (max8 at 1 elem/lane/cycle) is the pacing engine: ~1.37us x 65 units.
Queries are host-pre-transposed; 1/|q| and 1/(441*3) live in the host-built
amask, applied by the per-way score matmuls; host sums m-tiles and cores.
"""
import os
import sys

import numpy as np

for _p in ('/opt/trn_rl_repo', '/root/.axon_site/_ro/trn_rl_repo'):
    if os.path.isdir(_p) and _p not in sys.path:
        sys.path.insert(0, _p)

WAYS, SHOTS, Q = 5, 5, 30
C, HW = 128, 441
K = 3
NWAY = SHOTS * HW            # 2205 support descriptors per way
WPAD = 2208                  # per-way padded width (3 zero descriptors)
HALF = WPAD // 2             # 1104 pairs per way
ND = WAYS * WPAD             # 11040
DT = 87                      # bank column-tiles of 128
ND_PAD = DT * 128            # 11136
NCORES = 8
TROWS = Q * HW               # 13230 query-descriptor rows in total
RPC = (TROWS + NCORES - 1) // NCORES   # 1654 rows per core
MT = (RPC + 127) // 128      # 13 m-tiles per core
M_PAD = MT * 128             # 1664
SLOTS = 8                    # local query slots a core's rows can span (<=5)

QUART = HALF // 2            # 552

# one packed input tensor, staged dma_starts (each ~2.5us fixed):
# [zqt_t0 | bank_way0 | zqt_rest | bank_rest | amask16]
OFF_ZQT0 = 0
OFF_BANK0 = OFF_ZQT0 + 128
OFF_ZQTR = OFF_BANK0 + WPAD
OFF_BANKR = OFF_ZQTR + (MT - 1) * 128
OFF_AM = OFF_BANKR + (ND_PAD - WPAD)
BLOB = OFF_AM + 2 * MT * SLOTS

_CACHE = {}


def _build_program():
    import concourse.bacc as bacc
    import concourse.mybir as mybir
    from concourse import tile

    dt = mybir.dt
    AF = mybir.ActivationFunctionType
    ALU = mybir.AluOpType
    AX = mybir.AxisListType

    nc = bacc.Bacc('TRN2', target_bir_lowering=False, debug=False)

    d_blob = nc.dram_tensor('blob', [128, BLOB], dt.float16, kind='ExternalInput')
    d_out = nc.dram_tensor('scores', [SLOTS, WAYS * MT], dt.float32,
                           kind='ExternalOutput')

    with tile.TileContext(nc) as tc:
        with tc.tile_pool(name='persist', bufs=1) as pp, \
             tc.tile_pool(name='work', bufs=3) as wp:

            blob = pp.tile([128, BLOB], dt.float16)

            def zqt(t):
                if t == 0:
                    return blob[:, OFF_ZQT0:OFF_ZQT0 + 128]
                o = OFF_ZQTR + (t - 1) * 128
                return blob[:, o:o + 128]

            def bankw(w, lo, hi):
                if w == 0:
                    return blob[:, OFF_BANK0 + lo:OFF_BANK0 + hi]
                o = OFF_BANKR + (w - 1) * WPAD
                return blob[:, o + lo:o + hi]

            amask3 = blob[:, OFF_AM:OFF_AM + 2 * MT * SLOTS].bitcast(
                dt.float32).rearrange('p (t s) -> p t s', t=MT)
            scw = pp.tile([SLOTS, WAYS, MT], dt.float32)

            # ---- input DMAs, staged so unit (0,0) starts asap ----
            nc.sync.dma_start(blob[:, 0:OFF_BANK0 + HALF],
                              d_blob[:, 0:OFF_BANK0 + HALF])
            nc.sync.dma_start(blob[:, OFF_BANK0 + HALF:OFF_ZQTR],
                              d_blob[:, OFF_BANK0 + HALF:OFF_ZQTR])
            nc.sync.dma_start(blob[:, OFF_ZQTR:OFF_BANKR],
                              d_blob[:, OFF_ZQTR:OFF_BANKR])
            nc.sync.dma_start(blob[:, OFF_BANKR:BLOB],
                              d_blob[:, OFF_BANKR:BLOB])

            with tc.tile_pool(name='ps', bufs=4, space='PSUM') as ps:

                halves = [(w, t, h) for w in range(WAYS) for t in range(MT)
                          for h in range(2)]
                m8bigs, m16s, pRs = {}, {}, {}
                pending = []

                def emit_front(i):
                    w, t, h = halves[i]
                    pR = ps.tile([128, QUART], dt.float32, tag='pR',
                                 name=f'pR_{i}')
                    pRs[i] = pR
                    base = h * QUART
                    for off, sz in ((0, 512), (512, 40)):
                        nc.tensor.matmul(pR[:, off:off + sz], zqt(t),
                                         bankw(w, base + off, base + off + sz),
                                         start=True, stop=True)
                    nc.scalar.activation(pR[:], pR[:], AF.Relu)

                def emit_back(i):
                    w, t, h = halves[i]
                    pR = pRs.pop(i)
                    if t == 0 and h == 0:
                        m8bigs[w] = wp.tile([128, MT, 8], dt.float32, tag='m8',
                                            name=f'm8_{w}')
                    if h == 0:
                        m16s[w, t] = wp.tile([128, 2, 8], dt.float32,
                                             tag='m16', name=f'm16_{i}')
                    base = HALF + h * QUART
                    # b-sims accumulate onto relu(delta): pR = max(a, b)
                    for off, sz in ((0, 512), (512, 40)):
                        nc.tensor.matmul(pR[:, off:off + sz], zqt(t),
                                         bankw(w, base + off, base + off + sz),
                                         start=False, stop=True)
                    nc.vector.max(m16s[w, t][:, h, :], pR[:])
                    if h == 1:
                        m16 = m16s.pop((w, t))
                        nc.vector.max(m8bigs[w][:, t, :],
                                      m16[:].rearrange('p a b -> p (a b)'))
                        if t == MT - 1:
                            pending.append(w)

                def emit_wayend():
                    w = pending.pop(0)
                    m8big = m8bigs.pop(w)
                    stv = wp.tile([128, MT], dt.float32, tag='stv')
                    nc.vector.reduce_sum(stv[:], m8big[:, :, 0:K], axis=AX.X)
                    # borrow a rotating psum tile for this way's 13 tiny
                    # score matmuls, then stash the [SLOTS, MT] result in SBUF
                    sc = ps.tile([128, QUART], dt.float32, tag='pR',
                                 name=f'sc_{w}')
                    for tt in range(MT):
                        nc.tensor.matmul(sc[0:SLOTS, tt:tt + 1],
                                         amask3[:, tt, :], stv[:, tt:tt + 1],
                                         start=True, stop=True)
                    nc.scalar.activation(scw[:, w, :], sc[0:SLOTS, 0:MT],
                                         AF.Copy)

                # software-pipelined so the in-order PE queue never waits on
                # a relu; 4 psum bufs keep ~3 half-units in flight
                for i in range(len(halves) + 1):
                    if i < len(halves):
                        emit_front(i)
                    if i >= 1:
                        emit_back(i - 1)
                    if pending and (i - 1 >= len(halves) - 1 or
                                    (i >= 8 and halves[i - 8][1] == MT - 1
                                     and halves[i - 8][2] == 1)):
                        emit_wayend()
                while pending:
                    emit_wayend()
            # host sums the MT axis (and across cores)
            nc.sync.dma_start(d_out[:], scw[:].rearrange('s w t -> s (w t)'))

    nc.finalize()
    return nc


def _host_prep(support_images, support_labels, query_images):
    support_images = np.ascontiguousarray(np.asarray(support_images, np.float32))
    support_labels = np.asarray(support_labels, np.float32)
    query_images = np.ascontiguousarray(np.asarray(query_images, np.float32))

    labels = np.argmax(support_labels, axis=1)
    order = np.argsort(labels, kind='stable')
    sup = support_images[order].reshape(WAYS * SHOTS, C, HW)

    # replicated class-descriptor bank: grouped, fp16, L2-normalized over C
    # (norms from the fp16-rounded values the matmuls see), padded per way
    desc = sup.transpose(0, 2, 1).reshape(WAYS, NWAY, C).astype(np.float16)
    dn = np.sqrt((desc.astype(np.float32) ** 2).sum(-1, keepdims=True) + 1e-4)
    dhat = (desc.astype(np.float32) / dn)
    dpad = np.zeros((WAYS, WPAD, C), np.float32)
    dpad[:, :NWAY] = dhat
    # [delta | b] halves per way
    bankw = np.empty_like(dpad)
    bankw[:, :HALF] = dpad[:, :HALF] - dpad[:, HALF:]
    bankw[:, HALF:] = dpad[:, HALF:]
    flat = bankw.reshape(ND, C)
    flat = np.concatenate([flat, np.zeros((ND_PAD - ND, C), np.float32)], 0)
    bank_dev = flat.T.astype(np.float16)                         # [C, ND_PAD]

    # flat query-descriptor rows [13230, C], row r = (q = r//441, hw = r%441)
    zq_flat = query_images.reshape(Q, C, HW).transpose(0, 2, 1).reshape(TROWS, C)
    blob_devs = []
    for core in range(NCORES):
        r0 = core * RPC
        zb = zq_flat[r0:r0 + RPC]
        zb = np.concatenate(
            [zb, np.zeros((M_PAD - zb.shape[0], C), np.float32)], 0)
        zqt_dev = zb.T.reshape(C, MT * 128).astype(np.float16)
        # 1/|q| per padded row (from the fp16 values the matmuls see),
        # folded into the amask weights
        q16 = zb.astype(np.float16).astype(np.float32)
        qn = np.sqrt((q16 ** 2).sum(1) + 1e-4)
        q0 = r0 // HW
        amask = np.zeros((128, MT, SLOTS), np.float32)
        lr = np.arange(MT * 128)
        r = r0 + lr
        valid = (lr < RPC) & (r < TROWS)
        amask[lr[valid] % 128, lr[valid] // 128, (r[valid] // HW) - q0] = \
            1.0 / (HW * K * qn[lr[valid]])
        am16 = amask.reshape(128, MT * SLOTS).view(np.float16)
        blob = np.concatenate(
            [zqt_dev[:, 0:128], bank_dev[:, 0:WPAD], zqt_dev[:, 128:],
             bank_dev[:, WPAD:], am16], axis=1)
        blob_devs.append(np.ascontiguousarray(blob))
    return blob_devs


def kernel(support_images, support_labels, query_images):
    from concourse import bass_utils

    if 'nc' not in _CACHE:
        _CACHE['nc'] = _build_program()
    nc = _CACHE['nc']

    blob_devs = _host_prep(support_images, support_labels, query_images)

    in_maps = [{'blob': blob_devs[c]} for c in range(NCORES)]
    try:
        res = bass_utils.run_bass_kernel_spmd(
            nc, in_maps, core_ids=list(range(NCORES)))
    except Exception:
        # transient NRT/tunnel failures happen; one retry
        import time
        time.sleep(2.0)
        res = bass_utils.run_bass_kernel_spmd(
            nc, in_maps, core_ids=list(range(NCORES)))
    scores = np.zeros((Q, WAYS), np.float32)
    for c in range(NCORES):
        q0 = (c * RPC) // HW
        part = res.results[c]['scores'].reshape(SLOTS, WAYS, MT).sum(2)
        for s in range(SLOTS):
            if q0 + s < Q:
                scores[q0 + s] += part[s]
    return scores.astype(np.float32)


# revision 7
# speedup vs baseline: 1.7279x; 1.0133x over previous
"""DN4 retrieval-kNN kernel for Trainium2 (8 NeuronCores, SPMD, no collectives).

v5: relu-fold with PSUM accumulation, half-unit pipelining. Host prepares the replicated
class-descriptor bank (grouped, L2-normalized, transposed to [C, n]) with
each way's 2208 padded columns stored as [delta | b]: delta_j =
d_j - d_{j+1104}, b_j = d_{j+1104}. On device, per (way, m-tile) unit:

  PE:  delta-sims -> pR psum fp32                 (q . delta, 1104 cols)
  ACT: relu(pR) -> pR IN PLACE                    (one pass, half the old)
  PE:  b-sims ACCUMULATE onto pR (start=False)    -> pR = b + relu(a-b)
                                                   = max(a, b) exactly
  DVE: max8 over the 1104 pair-maxes in pR, top-3 of those == top-3 of
       the way's 2205 sims unless >=2 of the top-3 share a pair
       (P ~ 3/1104 per row; error way under the 2e-2 tolerance)

DVE (max8 at 1 elem/lane/cycle) is the pacing engine: ~1.37us x 65 units.
Queries are host-pre-transposed; 1/|q| and 1/(441*3) live in the host-built
amask, applied by the per-way score matmuls; host sums m-tiles and cores.
"""
import os
import sys

import numpy as np

for _p in ('/opt/trn_rl_repo', '/root/.axon_site/_ro/trn_rl_repo'):
    if os.path.isdir(_p) and _p not in sys.path:
        sys.path.insert(0, _p)

WAYS, SHOTS, Q = 5, 5, 30
C, HW = 128, 441
K = 3
NWAY = SHOTS * HW            # 2205 support descriptors per way
WPAD = 2208                  # per-way padded width (3 zero descriptors)
HALF = WPAD // 2             # 1104 pairs per way
ND = WAYS * WPAD             # 11040
DT = 87                      # bank column-tiles of 128
ND_PAD = DT * 128            # 11136
NCORES = 8
TROWS = Q * HW               # 13230 query-descriptor rows in total
RPC = (TROWS + NCORES - 1) // NCORES   # 1654 rows per core
MT = (RPC + 127) // 128      # 13 m-tiles per core
M_PAD = MT * 128             # 1664
SLOTS = 8                    # local query slots a core's rows can span (<=5)

QUART = HALF // 2            # 552

# one packed input tensor, staged dma_starts (each ~2.5us fixed):
# [zqt_t0 | bank_way0 | zqt_rest | bank_rest | amask16]
OFF_ZQT0 = 0
OFF_BANK0 = OFF_ZQT0 + 128
OFF_ZQTR = OFF_BANK0 + WPAD
OFF_BANKR = OFF_ZQTR + (MT - 1) * 128
OFF_AM = OFF_BANKR + (ND_PAD - WPAD)
BLOB = OFF_AM + 2 * MT * SLOTS

_CACHE = {}


def _build_program():
    import concourse.bacc as bacc
    import concourse.mybir as mybir
    from concourse import tile

    dt = mybir.dt
    AF = mybir.ActivationFunctionType
    ALU = mybir.AluOpType
    AX = mybir.AxisListType

    nc = bacc.Bacc('TRN2', target_bir_lowering=False, debug=False)

    d_blob = nc.dram_tensor('blob', [128, BLOB], dt.float16, kind='ExternalInput')
    d_out = nc.dram_tensor('scores', [SLOTS, WAYS * MT], dt.float32,
                           kind='ExternalOutput')

    with tile.TileContext(nc) as tc:
        with tc.tile_pool(name='persist', bufs=1) as pp, \
             tc.tile_pool(name='work', bufs=3) as wp:

            blob = pp.tile([128, BLOB], dt.float16)

            def zqt(t):
                if t == 0:
                    return blob[:, OFF_ZQT0:OFF_ZQT0 + 128]
                o = OFF_ZQTR + (t - 1) * 128
                return blob[:, o:o + 128]

            def bankw(w, lo, hi):
                if w == 0:
                    return blob[:, OFF_BANK0 + lo:OFF_BANK0 + hi]
                o = OFF_BANKR + (w - 1) * WPAD
                return blob[:, o + lo:o + hi]

            amask3 = blob[:, OFF_AM:OFF_AM + 2 * MT * SLOTS].bitcast(
                dt.float32).rearrange('p (t s) -> p t s', t=MT)
            scw = pp.tile([SLOTS, WAYS, MT], dt.float32)

            # ---- input DMAs, staged so unit (0,0) starts asap ----
            nc.sync.dma_start(blob[:, 0:OFF_BANK0 + HALF],
                              d_blob[:, 0:OFF_BANK0 + HALF])
            nc.sync.dma_start(blob[:, OFF_BANK0 + HALF:OFF_ZQTR],
                              d_blob[:, OFF_BANK0 + HALF:OFF_ZQTR])
            nc.sync.dma_start(blob[:, OFF_ZQTR:OFF_BANKR],
                              d_blob[:, OFF_ZQTR:OFF_BANKR])
            nc.sync.dma_start(blob[:, OFF_BANKR:BLOB],
                              d_blob[:, OFF_BANKR:BLOB])

            with tc.tile_pool(name='ps', bufs=4, space='PSUM') as ps:

                halves = [(w, t, h) for w in range(WAYS) for t in range(MT)
                          for h in range(2)]
                # warm the PE through its p-state ramp during the input DMA
                junk16 = pp.tile([128, C], dt.float16, name='junk16w')
                nc.gpsimd.memset(junk16[:], 0.0)
                warm = ps.tile([128, QUART], dt.float32, tag='pR',
                               name='warm')
                for _ in range(12):
                    nc.tensor.matmul(warm[:, 0:128], junk16[:], junk16[:],
                                     start=True, stop=True)
                m8bigs, m16s, pRs = {}, {}, {}
                pending = []

                def emit_front(i):
                    w, t, h = halves[i]
                    pR = ps.tile([128, QUART], dt.float32, tag='pR',
                                 name=f'pR_{i}')
                    pRs[i] = pR
                    base = h * HALF
                    for off, sz in ((0, 512), (512, 40)):
                        nc.tensor.matmul(pR[:, off:off + sz], zqt(t),
                                         bankw(w, base + off, base + off + sz),
                                         start=True, stop=True)
                    nc.scalar.activation(pR[:], pR[:], AF.Relu)

                def emit_back(i):
                    w, t, h = halves[i]
                    pR = pRs.pop(i)
                    if t == 0 and h == 0:
                        m8bigs[w] = wp.tile([128, MT, 8], dt.float32, tag='m8',
                                            name=f'm8_{w}')
                    if h == 0:
                        m16s[w, t] = wp.tile([128, 2, 8], dt.float32,
                                             tag='m16', name=f'm16_{i}')
                    base = h * HALF + QUART
                    # b-sims accumulate onto relu(delta): pR = max(a, b)
                    for off, sz in ((0, 512), (512, 40)):
                        nc.tensor.matmul(pR[:, off:off + sz], zqt(t),
                                         bankw(w, base + off, base + off + sz),
                                         start=False, stop=True)
                    nc.vector.max(m16s[w, t][:, h, :], pR[:])
                    if h == 1:
                        m16 = m16s.pop((w, t))
                        nc.vector.max(m8bigs[w][:, t, :],
                                      m16[:].rearrange('p a b -> p (a b)'))
                        if t == MT - 1:
                            pending.append(w)

                def emit_wayend():
                    w = pending.pop(0)
                    m8big = m8bigs.pop(w)
                    stv = wp.tile([128, MT], dt.float32, tag='stv')
                    nc.vector.reduce_sum(stv[:], m8big[:, :, 0:K], axis=AX.X)
                    # borrow a rotating psum tile for this way's 13 tiny
                    # score matmuls, then stash the [SLOTS, MT] result in SBUF
                    sc = ps.tile([128, QUART], dt.float32, tag='pR',
                                 name=f'sc_{w}')
                    for tt in range(MT):
                        nc.tensor.matmul(sc[0:SLOTS, tt:tt + 1],
                                         amask3[:, tt, :], stv[:, tt:tt + 1],
                                         start=True, stop=True)
                    nc.scalar.activation(scw[:, w, :], sc[0:SLOTS, 0:MT],
                                         AF.Copy)

                # software-pipelined so the in-order PE queue never waits on
                # a relu; 4 psum bufs keep ~3 half-units in flight
                for i in range(len(halves) + 1):
                    if i < len(halves):
                        emit_front(i)
                    if i >= 1:
                        emit_back(i - 1)
                    if pending and (i - 1 >= len(halves) - 1 or
                                    (i >= 8 and halves[i - 8][1] == MT - 1
                                     and halves[i - 8][2] == 1)):
                        emit_wayend()
                while pending:
                    emit_wayend()
            # host sums the MT axis (and across cores)
            nc.sync.dma_start(d_out[:], scw[:].rearrange('s w t -> s (w t)'))

    nc.finalize()
    return nc


def _host_prep(support_images, support_labels, query_images):
    support_images = np.ascontiguousarray(np.asarray(support_images, np.float32))
    support_labels = np.asarray(support_labels, np.float32)
    query_images = np.ascontiguousarray(np.asarray(query_images, np.float32))

    labels = np.argmax(support_labels, axis=1)
    order = np.argsort(labels, kind='stable')
    sup = support_images[order].reshape(WAYS * SHOTS, C, HW)

    # replicated class-descriptor bank: grouped, fp16, L2-normalized over C
    # (norms from the fp16-rounded values the matmuls see), padded per way
    desc = sup.transpose(0, 2, 1).reshape(WAYS, NWAY, C).astype(np.float16)
    dn = np.sqrt((desc.astype(np.float32) ** 2).sum(-1, keepdims=True) + 1e-4)
    dhat = (desc.astype(np.float32) / dn)
    dpad = np.zeros((WAYS, WPAD, C), np.float32)
    dpad[:, :NWAY] = dhat
    # [delta_h0 | b_h0 | delta_h1 | b_h1] per way: half-unit h covers pairs
    # (j, j+HALF) for j in [h*QUART, (h+1)*QUART)
    delta = dpad[:, :HALF] - dpad[:, HALF:]
    bvals = dpad[:, HALF:]
    bankw = np.concatenate(
        [delta[:, :QUART], bvals[:, :QUART],
         delta[:, QUART:], bvals[:, QUART:]], axis=1)
    flat = bankw.reshape(ND, C)
    flat = np.concatenate([flat, np.zeros((ND_PAD - ND, C), np.float32)], 0)
    bank_dev = flat.T.astype(np.float16)                         # [C, ND_PAD]

    # flat query-descriptor rows [13230, C], row r = (q = r//441, hw = r%441)
    zq_flat = query_images.reshape(Q, C, HW).transpose(0, 2, 1).reshape(TROWS, C)
    blob_devs = []
    for core in range(NCORES):
        r0 = core * RPC
        zb = zq_flat[r0:r0 + RPC]
        zb = np.concatenate(
            [zb, np.zeros((M_PAD - zb.shape[0], C), np.float32)], 0)
        zqt_dev = zb.T.reshape(C, MT * 128).astype(np.float16)
        # 1/|q| per padded row (from the fp16 values the matmuls see),
        # folded into the amask weights
        q16 = zb.astype(np.float16).astype(np.float32)
        qn = np.sqrt((q16 ** 2).sum(1) + 1e-4)
        q0 = r0 // HW
        amask = np.zeros((128, MT, SLOTS), np.float32)
        lr = np.arange(MT * 128)
        r = r0 + lr
        valid = (lr < RPC) & (r < TROWS)
        amask[lr[valid] % 128, lr[valid] // 128, (r[valid] // HW) - q0] = \
            1.0 / (HW * K * qn[lr[valid]])
        am16 = amask.reshape(128, MT * SLOTS).view(np.float16)
        blob = np.concatenate(
            [zqt_dev[:, 0:128], bank_dev[:, 0:WPAD], zqt_dev[:, 128:],
             bank_dev[:, WPAD:], am16], axis=1)
        blob_devs.append(np.ascontiguousarray(blob))
    return blob_devs


def kernel(support_images, support_labels, query_images):
    from concourse import bass_utils

    if 'nc' not in _CACHE:
        _CACHE['nc'] = _build_program()
    nc = _CACHE['nc']

    blob_devs = _host_prep(support_images, support_labels, query_images)

    in_maps = [{'blob': blob_devs[c]} for c in range(NCORES)]
    try:
        res = bass_utils.run_bass_kernel_spmd(
            nc, in_maps, core_ids=list(range(NCORES)))
    except Exception:
        # transient NRT/tunnel failures happen; one retry
        import time
        time.sleep(2.0)
        res = bass_utils.run_bass_kernel_spmd(
            nc, in_maps, core_ids=list(range(NCORES)))
    scores = np.zeros((Q, WAYS), np.float32)
    for c in range(NCORES):
        q0 = (c * RPC) // HW
        part = res.results[c]['scores'].reshape(SLOTS, WAYS, MT).sum(2)
        for s in range(SLOTS):
            if q0 + s < Q:
                scores[q0 + s] += part[s]
    return scores.astype(np.float32)
